# Initial kernel scaffold
#
"""D-FPS (furthest point sampling) on Trainium2 via Bass — self-contained kernel.

Contract: kernel(points=[8,131072,3] f32, features=[8,16,131072] f32 (unused),
npoint=1024) -> [8,1024] int32 indices, matching the jax reference bit-exactly.

Sharding: data-parallel over batch — each of the 8 NeuronCores runs FPS on its
own point cloud, fully SBUF-resident (points planes + min-distance buffer kept
on-chip across all npoint iterations).

Per FPS iteration (all fp32 arithmetic in reference order, on the DVE):
  op1 FPS_SQ2      acc   = (x - qx)^2 + (y - qy)^2
  op2 FPS_SQACC    d     = (z - qz)^2 + acc          (accum max -> v at t=1)
  op3 FPS_MINMAX   min_d = min(min_d, d); accum max -> v
  op4 FPS_ARGMAXENC R    = max over {k : min_d[k] == v} of (CB1[p] - k)
                    with CB1[p] = N - p*NF  =>  R = N - flat_first_argmax
Cross-partition winner: PE transposes of v/R columns -> rows on partition 0;
max8/max_index -> winning partition p*; PE sequencer registers compute
flat/i* from R*; PE matmuls (identity one-hot passthroughs, static stationary
operands only) gather the winner's (x,y,z) and broadcast to all partitions.
Indices are accumulated as R* history and converted to flat indices once.
"""

import sys

sys.path.insert(0, "/opt/trn_rl_repo")

import numpy as np

import concourse.bacc as bacc
import concourse.mybir as mybir
import concourse.tile as tile
from concourse.bass import ds
from concourse.masks import make_identity
from concourse.dve_uop import DveOpSpec
from concourse import dve_ops
from concourse.dve_spec import (
    Spec,
    Src0,
    Src1,
    C0,
    C1,
    Zero,
    sq,
    select,
    minn,
    Idx,
    lower,
    AluOp,
    _has_src1,
)

FP32 = mybir.dt.float32
I32 = mybir.dt.int32
U32 = mybir.dt.uint32
P = 128


def _register_op(name, spec):
    if name in dve_ops._SUB_OPCODE_FOR_NAME:
        for op in dve_ops.OPS:
            if op.name == name:
                return op
        raise RuntimeError(f"{name} in row map but not OPS")
    row = dve_ops._CUSTOM_DVE_ROW_BASE + len(dve_ops.OPS)
    assert row < 0x20, "custom DVE row overflow"
    dve_ops._SUB_OPCODE_FOR_NAME[name] = row
    shas = {}
    for ver in ("v3", "v4"):
        try:
            s = DveOpSpec(
                name=name, opcode=row, uops=lower(spec, ver=ver), rd1_en=_has_src1(spec)
            )
            shas[ver] = s.sha(ver)
        except Exception:
            if ver == "v3":
                raise
    op = dve_ops.DveOp(name, spec, subdim=False, uops_sha=shas)
    dve_ops.OPS.append(op)
    dve_ops.CUSTOM_DVE_SPECS[name] = spec
    return op


def _ref_sq2(in0, in1, s0, s1, imm2):
    a = (in0.astype(np.float32) - s0) ** 2
    b = (in1.astype(np.float32) - s1) ** 2
    return (a + b).astype(np.float32)


def _ref_sqacc(in0, in1, s0, s1, imm2):
    b = ((in0.astype(np.float32) - s0) ** 2 + in1).astype(np.float32)
    r = b.reshape(b.shape[0], -1).max(axis=-1, keepdims=True)
    return b, np.maximum(r, -np.finfo(np.float32).max)


def _ref_minmax(in0, in1, s0, s1, imm2):
    b = np.minimum(in0.astype(np.float32), in1.astype(np.float32))
    r = b.reshape(b.shape[0], -1).max(axis=-1, keepdims=True)
    return b, np.maximum(r, np.float32(0.0))


def _ref_argmaxenc(in0, in1, s0, s1, imm2):
    x = in0.astype(np.float32)
    flat = x.reshape(x.shape[0], -1)
    n = flat.shape[1]
    idx = np.arange(n, dtype=np.float32)
    sel = np.where(flat >= s0, (s1 - idx).astype(np.float32), np.float32(0.0)).astype(
        np.float32
    )
    r = sel.max(axis=-1, keepdims=True)
    return sel.reshape(x.shape), np.maximum(r, np.float32(0.0))


FPS_SQ2 = _register_op(
    "FPS_SQ2_ANT", Spec(body=sq(Src0 - C0) + sq(Src1 - C1), reference=_ref_sq2)
)
FPS_SQACC = _register_op(
    "FPS_SQACC_ANT",
    Spec(body=sq(Src0 - C0) + Src1, accum=AluOp.MAX, reference=_ref_sqacc),
)
FPS_MINMAX = _register_op(
    "FPS_MINMAX_ANT",
    Spec(
        body=minn(Src0, Src1),
        accum=AluOp.MAX,
        accum_init=Zero,
        reference=_ref_minmax,
    ),
)
FPS_ARGMAXENC = _register_op(
    "FPS_ARGMAXENC_ANT",
    Spec(
        body=select(Src0 >= C0, C1 - Idx, Zero),
        accum=AluOp.MAX,
        accum_init=Zero,
        reference=_ref_argmaxenc,
    ),
)


def build_fps_nc(NF: int, NPOINT: int, debug: bool = False):
    """Input 'points' [128, 3*NF] fp32 (= [N,3] row-major reshaped),
    output 'indices' [1, NPOINT] int32."""
    N = P * NF
    nc = bacc.Bacc("TRN2", target_bir_lowering=False, debug=debug)

    pts_in = nc.dram_tensor("points", [P, 3 * NF], FP32, kind="ExternalInput")
    idx_out = nc.dram_tensor("indices", [1, NPOINT], I32, kind="ExternalOutput")

    with tile.TileContext(nc) as tc:
        with (
            tc.tile_pool(name="persist", bufs=1) as pool,
            tc.tile_pool(name="psum", bufs=1, space="PSUM") as psum,
        ):
            planes = pool.tile([P, 3 * NF], FP32, tag="planes")
            min_d = pool.tile([P, NF], FP32, tag="min_d")
            acc = pool.tile([P, NF], FP32, tag="acc")
            dbuf = pool.tile([P, NF], FP32, tag="dbuf")
            vcol = pool.tile([P, 1], FP32, tag="vcol")
            rcol = pool.tile([P, 1], FP32, tag="rcol")
            vdummy = pool.tile([P, 1], FP32, tag="vdummy")
            vS = pool.tile([1, P], FP32, tag="vS")
            ru_row = pool.tile([1, P], U32, tag="ru_row")
            m8 = pool.tile([1, 8], FP32, tag="m8")
            pi8 = pool.tile([1, 8], U32, tag="pi8")
            q_s0 = pool.tile([P, 3], FP32, tag="q_s0")
            out_sb = pool.tile([1, NPOINT], I32, tag="out_sb")
            rhist = pool.tile([1, NPOINT], I32, tag="rhist")
            ident = pool.tile([P, P], FP32, tag="ident")
            cb1 = pool.tile([P, 1], I32, tag="cb1")
            cb1f = pool.tile([P, 1], FP32, tag="cb1f")
            ones_row = pool.tile([1, P], FP32, tag="ones_row")

            vT_ps = psum.tile([1, P], FP32, tag="vT_ps")
            rT_ps = psum.tile([1, P], FP32, tag="rT_ps")
            qb_ps = psum.tile([P, 3], FP32, tag="qb_ps")
            candcol_ps = psum.tile([P, 3], FP32, tag="candcol_ps")
            qcol_ps = psum.tile([3, 1], FP32, tag="qcol_ps")
            candcol = pool.tile([P, 3], FP32, tag="candcol")
            qcol3 = pool.tile([3, 1], FP32, tag="qcol3")
            qrow_sb = pool.tile([1, 3], FP32, tag="qrow_sb")

            # ---- init ----
            make_identity(nc, ident)
            nc.gpsimd.iota(cb1, pattern=[[0, 1]], base=N, channel_multiplier=-NF)
            nc.vector.tensor_copy(cb1f[:], cb1[:])  # int32 -> fp32 (exact)
            nc.vector.memset(out_sb[0:1, 0:1], 0)
            nc.vector.memset(ones_row[:], 1.0)

            with tc.tile_pool(name="initbuf", bufs=1) as ipool:
                buf_i = ipool.tile([P, 3 * NF], FP32, tag="buf_i")
                nc.sync.dma_start(buf_i[:], pts_in[:])
                b3 = buf_i.rearrange("p (f c) -> p f c", c=3)
                for c in range(3):
                    nc.vector.tensor_copy(planes[:, c * NF : (c + 1) * NF], b3[:, :, c])

            planes3 = planes.rearrange("p (c f) -> p c f", c=3)

            # q for t=0 is point 0: [1,3] row on partition 0, broadcast via PE.
            nc.tensor.matmul(
                out=qb_ps[:, 0:3], lhsT=ones_row[0:1, :], rhs=planes3[0:1, :, 0]
            )
            nc.vector.tensor_copy(q_s0[:], qb_ps[:, 0:3])

            x_pl = planes[:, 0:NF]
            y_pl = planes[:, NF : 2 * NF]
            z_pl = planes[:, 2 * NF : 3 * NF]

            regs_pe = (
                nc.tensor.register("rp"),
                nc.tensor.register("rt"),
                nc.tensor.register("ri"),
                nc.tensor.register("rm"),
                nc.tensor.register("rc0"),
                nc.tensor.register("rc1"),
                nc.tensor.register("rc2"),
            )
            rp, rt, ri, rm, rc0, rc1, rc2 = (r.__enter__() for r in regs_pe)

            for t in range(1, NPOINT):
                first = t == 1
                nc.vector._custom_dve(
                    FPS_SQ2,
                    out=acc[:],
                    in0=x_pl,
                    in1=y_pl,
                    s0=q_s0[:, 0:1],
                    s1=q_s0[:, 1:2],
                )
                nc.vector._custom_dve(
                    FPS_SQACC,
                    out=(min_d[:] if first else dbuf[:]),
                    accum_out=(vcol[:] if first else vdummy[:]),
                    in0=z_pl,
                    in1=acc[:],
                    s0=q_s0[:, 2:3],
                )
                if not first:
                    nc.vector._custom_dve(
                        FPS_MINMAX,
                        out=min_d[:],
                        accum_out=vcol[:],
                        in0=min_d[:],
                        in1=dbuf[:],
                    )
                nc.vector._custom_dve(
                    FPS_ARGMAXENC,
                    out=acc[:],
                    accum_out=rcol[:],
                    in0=min_d[:],
                    s0=vcol[:],
                    s1=cb1f[:],
                )
                # --- cross-partition winner ---
                nc.tensor.transpose(vT_ps[:], vcol[:], ident[:])
                nc.tensor.transpose(rT_ps[:], rcol[:], ident[:])
                nc.vector.tensor_copy(vS[:], vT_ps[:])
                nc.scalar.copy(ru_row[0:1, :], rT_ps[0:1, :])  # fp32->u32 on ACT
                nc.vector.max(out=m8[:], in_=vS[:])
                nc.vector.max_index(out=pi8[:], in_max=m8[:], in_values=vS[:])

                # --- PE: winner coords -> broadcast; R* -> rhist ---
                nc.tensor.reg_load(rp, pi8[0:1, 0:1])
                rp_v = nc.tensor.snap(rp, donate=True, min_val=0, max_val=P - 1)
                nc.tensor.reg_load(rt, ru_row[0:1, ds(rp_v, 1)])
                nc.tensor.reg_mov(ri, N)
                nc.tensor.reg_alu(ri, ri, rt, mybir.AluOpType.subtract)
                nc.tensor.reg_alu(rm, rp, NF, mybir.AluOpType.mult)
                nc.tensor.reg_alu(ri, ri, rm, mybir.AluOpType.subtract)
                nc.tensor.reg_save(rhist[0:1, t : t + 1], rt)
                ri_v = nc.tensor.snap(ri, donate=True, min_val=0, max_val=NF - 1)
                nc.tensor.matmul(
                    out=candcol_ps[:],
                    lhsT=ident[:],
                    rhs=planes3[:, :, ds(ri_v, 1)],
                )
                nc.vector.tensor_copy(candcol[:], candcol_ps[:])
                nc.tensor.matmul(
                    out=qcol_ps[:],
                    lhsT=candcol[:],
                    rhs=ident[:, ds(rp_v, 1)],
                )
                nc.vector.tensor_copy(qcol3[:], qcol_ps[:])
                q_u = qcol3[:].bitcast(I32)
                qr_u = qrow_sb[:].bitcast(I32)
                nc.tensor.reg_load(rc0, q_u[0:1, 0:1])
                nc.tensor.reg_load(rc1, q_u[1:2, 0:1])
                nc.tensor.reg_load(rc2, q_u[2:3, 0:1])
                nc.tensor.reg_save(qr_u[0:1, 0:1], rc0)
                nc.tensor.reg_save(qr_u[0:1, 1:2], rc1)
                nc.tensor.reg_save(qr_u[0:1, 2:3], rc2)
                nc.tensor.matmul(
                    out=qb_ps[:, 0:3],
                    lhsT=ones_row[0:1, :],
                    rhs=qrow_sb[0:1, 0:3],
                )
                nc.vector.tensor_copy(q_s0[:], qb_ps[:, 0:3])

            # indices = N - R (integer), once after the loop
            nc.vector.tensor_scalar(
                out_sb[0:1, 1:NPOINT],
                rhist[0:1, 1:NPOINT],
                -1,
                N,
                op0=mybir.AluOpType.mult,
                op1=mybir.AluOpType.add,
            )
            nc.sync.dma_start(idx_out[:], out_sb[:])

    nc.compile()
    return nc


_NC_CACHE = {}


def _get_nc(NF, NPOINT):
    key = (NF, NPOINT)
    if key not in _NC_CACHE:
        _NC_CACHE[key] = build_fps_nc(NF, NPOINT)
    return _NC_CACHE[key]


def kernel(points, features=None, npoint=1024, **_unused):
    """Full inputs in, full output out. points [B, N, 3] fp32 -> [B, npoint] i32."""
    from concourse.bass_utils import run_bass_kernel_spmd

    points = np.ascontiguousarray(np.asarray(points, dtype=np.float32))
    npoint = int(npoint)
    B, N, C = points.shape
    assert C == 3 and N % P == 0
    NF = N // P
    n_cores = 8
    assert B == n_cores, f"expected B == 8, got {B}"

    nc = _get_nc(NF, npoint)
    in_maps = [{"points": points[b].reshape(P, 3 * NF)} for b in range(B)]
    res = run_bass_kernel_spmd(nc, in_maps, list(range(n_cores)))
    out = np.stack(
        [np.asarray(res.results[b]["indices"]).ravel() for b in range(B)]
    ).astype(np.int32)
    return out


# revision 1
# speedup vs baseline: 1.1855x; 1.1855x over previous
"""D-FPS (furthest point sampling) on Trainium2 via Bass — self-contained kernel.

Contract: kernel(points=[8,131072,3] f32, features=[8,16,131072] f32 (unused),
npoint=1024) -> [8,1024] int32 indices, matching the jax reference bit-exactly.

Sharding: data-parallel over batch — each of the 8 NeuronCores runs FPS on its
own point cloud, fully SBUF-resident (points planes + min-distance buffer kept
on-chip across all npoint iterations).

Per FPS iteration (all fp32 arithmetic in reference order, on the DVE):
  op1 FPS_SQ2      acc   = (x - qx)^2 + (y - qy)^2
  op2 FPS_SQACC    d     = (z - qz)^2 + acc          (accum max -> v at t=1)
  op3 FPS_MINMAX   min_d = min(min_d, d); accum max -> v
  op4 FPS_ARGMAXENC R    = max over {k : min_d[k] == v} of (CB1[p] - k)
                    with CB1[p] = N - p*NF  =>  R = N - flat_first_argmax
Cross-partition winner: PE transposes of v/R columns -> rows on partition 0;
max8/max_index -> winning partition p*; PE sequencer registers compute
flat/i* from R*; PE matmuls (identity one-hot passthroughs, static stationary
operands only) gather the winner's (x,y,z) and broadcast to all partitions.
Indices are accumulated as R* history and converted to flat indices once.
"""

import sys

sys.path.insert(0, "/opt/trn_rl_repo")

import numpy as np

import concourse.bacc as bacc
import concourse.mybir as mybir
import concourse.tile as tile
from concourse.bass import ds
from concourse.masks import make_identity
from concourse.dve_uop import DveOpSpec
from concourse import dve_ops
from concourse.dve_spec import (
    Spec,
    Src0,
    Src1,
    C0,
    C1,
    Zero,
    sq,
    select,
    minn,
    Idx,
    lower,
    AluOp,
    _has_src1,
)

FP32 = mybir.dt.float32
I32 = mybir.dt.int32
U32 = mybir.dt.uint32
P = 128


def _register_op(name, spec):
    if name in dve_ops._SUB_OPCODE_FOR_NAME:
        for op in dve_ops.OPS:
            if op.name == name:
                return op
        raise RuntimeError(f"{name} in row map but not OPS")
    row = dve_ops._CUSTOM_DVE_ROW_BASE + len(dve_ops.OPS)
    assert row < 0x20, "custom DVE row overflow"
    dve_ops._SUB_OPCODE_FOR_NAME[name] = row
    shas = {}
    for ver in ("v3", "v4"):
        try:
            s = DveOpSpec(
                name=name, opcode=row, uops=lower(spec, ver=ver), rd1_en=_has_src1(spec)
            )
            shas[ver] = s.sha(ver)
        except Exception:
            if ver == "v3":
                raise
    op = dve_ops.DveOp(name, spec, subdim=False, uops_sha=shas)
    dve_ops.OPS.append(op)
    dve_ops.CUSTOM_DVE_SPECS[name] = spec
    return op


def _ref_sq2(in0, in1, s0, s1, imm2):
    a = (in0.astype(np.float32) - s0) ** 2
    b = (in1.astype(np.float32) - s1) ** 2
    return (a + b).astype(np.float32)


def _ref_sqacc(in0, in1, s0, s1, imm2):
    b = ((in0.astype(np.float32) - s0) ** 2 + in1).astype(np.float32)
    r = b.reshape(b.shape[0], -1).max(axis=-1, keepdims=True)
    return b, np.maximum(r, -np.finfo(np.float32).max)


def _ref_minmax(in0, in1, s0, s1, imm2):
    b = np.minimum(in0.astype(np.float32), in1.astype(np.float32))
    r = b.reshape(b.shape[0], -1).max(axis=-1, keepdims=True)
    return b, np.maximum(r, np.float32(0.0))


def _ref_argmaxenc(in0, in1, s0, s1, imm2):
    x = in0.astype(np.float32)
    flat = x.reshape(x.shape[0], -1)
    n = flat.shape[1]
    idx = np.arange(n, dtype=np.float32)
    sel = np.where(flat >= s0, (s1 - idx).astype(np.float32), np.float32(0.0)).astype(
        np.float32
    )
    r = sel.max(axis=-1, keepdims=True)
    return sel.reshape(x.shape), np.maximum(r, np.float32(0.0))


FPS_SQ2 = _register_op(
    "FPS_SQ2_ANT", Spec(body=sq(Src0 - C0) + sq(Src1 - C1), reference=_ref_sq2)
)
FPS_SQACC = _register_op(
    "FPS_SQACC_ANT",
    Spec(body=sq(Src0 - C0) + Src1, accum=AluOp.MAX, reference=_ref_sqacc),
)
FPS_MINMAX = _register_op(
    "FPS_MINMAX_ANT",
    Spec(
        body=minn(Src0, Src1),
        accum=AluOp.MAX,
        accum_init=Zero,
        reference=_ref_minmax,
    ),
)
FPS_ARGMAXENC = _register_op(
    "FPS_ARGMAXENC_ANT",
    Spec(
        body=select(Src0 >= C0, C1 - Idx, Zero),
        accum=AluOp.MAX,
        accum_init=Zero,
        reference=_ref_argmaxenc,
    ),
)


def build_fps_nc(NF: int, NPOINT: int, debug: bool = False):
    """Input 'points' [128, 3*NF] fp32 (= [N,3] row-major reshaped),
    output 'indices' [1, NPOINT] int32."""
    N = P * NF
    nc = bacc.Bacc("TRN2", target_bir_lowering=False, debug=debug)

    pts_in = nc.dram_tensor("points", [P, 3 * NF], FP32, kind="ExternalInput")
    idx_out = nc.dram_tensor("indices", [1, NPOINT], I32, kind="ExternalOutput")

    with tile.TileContext(nc) as tc:
        with (
            tc.tile_pool(name="persist", bufs=1) as pool,
            tc.tile_pool(name="psum", bufs=1, space="PSUM") as psum,
        ):
            planes = pool.tile([P, 3 * NF], FP32, tag="planes")
            min_d = pool.tile([P, NF], FP32, tag="min_d")
            acc = pool.tile([P, NF], FP32, tag="acc")
            dbuf = pool.tile([P, NF], FP32, tag="dbuf")
            vcol = pool.tile([P, 1], FP32, tag="vcol")
            rcol = pool.tile([P, 1], FP32, tag="rcol")
            vdummy = pool.tile([P, 1], FP32, tag="vdummy")
            vS = pool.tile([1, P], FP32, tag="vS")
            ru_row = pool.tile([1, P], U32, tag="ru_row")
            m8 = pool.tile([1, 8], FP32, tag="m8")
            pi8 = pool.tile([1, 8], U32, tag="pi8")
            q_s0 = pool.tile([P, 3], FP32, tag="q_s0")
            out_sb = pool.tile([1, NPOINT], I32, tag="out_sb")
            rhist = pool.tile([1, NPOINT], I32, tag="rhist")
            ident = pool.tile([P, P], FP32, tag="ident")
            cb1 = pool.tile([P, 1], I32, tag="cb1")
            cb1f = pool.tile([P, 1], FP32, tag="cb1f")
            ones_row = pool.tile([1, P], FP32, tag="ones_row")

            vT_ps = psum.tile([1, P], FP32, tag="vT_ps")
            rT_ps = psum.tile([1, P], FP32, tag="rT_ps")
            qb_ps = psum.tile([P, 3], FP32, tag="qb_ps")
            candcol_ps = psum.tile([P, 3], FP32, tag="candcol_ps")
            qcol_ps = psum.tile([3, 1], FP32, tag="qcol_ps")
            candcol = pool.tile([P, 3], FP32, tag="candcol")
            qcol3 = pool.tile([3, 1], FP32, tag="qcol3")
            qrow_sb = pool.tile([1, 3], FP32, tag="qrow_sb")

            # ---- init ----
            make_identity(nc, ident)
            nc.gpsimd.iota(cb1, pattern=[[0, 1]], base=N, channel_multiplier=-NF)
            nc.vector.tensor_copy(cb1f[:], cb1[:])  # int32 -> fp32 (exact)
            nc.vector.memset(out_sb[0:1, 0:1], 0)
            nc.vector.memset(ones_row[:], 1.0)

            with tc.tile_pool(name="initbuf", bufs=1) as ipool:
                buf_i = ipool.tile([P, 3 * NF], FP32, tag="buf_i")
                nc.sync.dma_start(buf_i[:], pts_in[:])
                b3 = buf_i.rearrange("p (f c) -> p f c", c=3)
                for c in range(3):
                    nc.vector.tensor_copy(planes[:, c * NF : (c + 1) * NF], b3[:, :, c])

            planes3 = planes.rearrange("p (c f) -> p c f", c=3)

            # q for t=0 is point 0: [1,3] row on partition 0, broadcast via PE.
            nc.tensor.matmul(
                out=qb_ps[:, 0:3], lhsT=ones_row[0:1, :], rhs=planes3[0:1, :, 0]
            )
            nc.vector.tensor_copy(q_s0[:], qb_ps[:, 0:3])

            x_pl = planes[:, 0:NF]
            y_pl = planes[:, NF : 2 * NF]
            z_pl = planes[:, 2 * NF : 3 * NF]

            regs_pe = (
                nc.tensor.register("rp"),
                nc.tensor.register("rt"),
                nc.tensor.register("ri"),
                nc.tensor.register("rm"),
                nc.tensor.register("rc0"),
                nc.tensor.register("rc1"),
                nc.tensor.register("rc2"),
            )
            rp, rt, ri, rm, rc0, rc1, rc2 = (r.__enter__() for r in regs_pe)

            for t in range(1, NPOINT):
                first = t == 1
                nc.vector._custom_dve(
                    FPS_SQ2,
                    out=acc[:],
                    in0=x_pl,
                    in1=y_pl,
                    s0=q_s0[:, 0:1],
                    s1=q_s0[:, 1:2],
                )
                nc.vector._custom_dve(
                    FPS_SQACC,
                    out=(min_d[:] if first else dbuf[:]),
                    accum_out=(vcol[:] if first else vdummy[:]),
                    in0=z_pl,
                    in1=acc[:],
                    s0=q_s0[:, 2:3],
                )
                if not first:
                    nc.vector._custom_dve(
                        FPS_MINMAX,
                        out=min_d[:],
                        accum_out=vcol[:],
                        in0=min_d[:],
                        in1=dbuf[:],
                    )
                nc.vector._custom_dve(
                    FPS_ARGMAXENC,
                    out=acc[:],
                    accum_out=rcol[:],
                    in0=min_d[:],
                    s0=vcol[:],
                    s1=cb1f[:],
                )
                # --- cross-partition winner ---
                nc.tensor.transpose(vT_ps[:], vcol[:], ident[:])
                nc.tensor.transpose(rT_ps[:], rcol[:], ident[:])
                nc.vector.tensor_copy(vS[:], vT_ps[:])
                nc.scalar.copy(ru_row[0:1, :], rT_ps[0:1, :])  # fp32->u32 on ACT
                nc.vector.max(out=m8[:], in_=vS[:])
                nc.vector.max_index(out=pi8[:], in_max=m8[:], in_values=vS[:])

                # --- PE: winner coords -> broadcast; R* -> rhist ---
                nc.tensor.reg_load(rp, pi8[0:1, 0:1])
                rp_v = nc.tensor.snap(rp, donate=True, min_val=0, max_val=P - 1)
                nc.tensor.reg_load(rt, ru_row[0:1, ds(rp_v, 1)])
                nc.tensor.reg_mov(ri, N)
                nc.tensor.reg_alu(ri, ri, rt, mybir.AluOpType.subtract)
                nc.tensor.reg_alu(rm, rp, NF, mybir.AluOpType.mult)
                nc.tensor.reg_alu(ri, ri, rm, mybir.AluOpType.subtract)
                nc.tensor.reg_save(rhist[0:1, t : t + 1], rt)
                ri_v = nc.tensor.snap(ri, donate=True, min_val=0, max_val=NF - 1)
                nc.tensor.matmul(
                    out=candcol_ps[:],
                    lhsT=ident[:],
                    rhs=planes3[:, :, ds(ri_v, 1)],
                )
                nc.vector.tensor_copy(candcol[:], candcol_ps[:])
                nc.tensor.matmul(
                    out=qcol_ps[:],
                    lhsT=candcol[:],
                    rhs=ident[:, ds(rp_v, 1)],
                )
                nc.vector.tensor_copy(qcol3[:], qcol_ps[:])
                q_u = qcol3[:].bitcast(I32)
                qr_u = qrow_sb[:].bitcast(I32)
                nc.tensor.reg_load(rc0, q_u[0:1, 0:1])
                nc.tensor.reg_load(rc1, q_u[1:2, 0:1])
                nc.tensor.reg_load(rc2, q_u[2:3, 0:1])
                nc.tensor.reg_save(qr_u[0:1, 0:1], rc0)
                nc.tensor.reg_save(qr_u[0:1, 1:2], rc1)
                nc.tensor.reg_save(qr_u[0:1, 2:3], rc2)
                nc.tensor.matmul(
                    out=qb_ps[:, 0:3],
                    lhsT=ones_row[0:1, :],
                    rhs=qrow_sb[0:1, 0:3],
                )
                nc.vector.tensor_copy(q_s0[:], qb_ps[:, 0:3])

            # indices = N - R (integer), once after the loop
            nc.vector.tensor_scalar(
                out_sb[0:1, 1:NPOINT],
                rhist[0:1, 1:NPOINT],
                -1,
                N,
                op0=mybir.AluOpType.mult,
                op1=mybir.AluOpType.add,
            )
            nc.sync.dma_start(idx_out[:], out_sb[:])

    nc.compile()
    return nc


_NC_CACHE = {}


def _get_nc(NF, NPOINT):
    key = (NF, NPOINT)
    if key not in _NC_CACHE:
        _NC_CACHE[key] = build_fps_nc(NF, NPOINT)
    return _NC_CACHE[key]


def kernel(points, features=None, npoint=1024, **_unused):
    """Full inputs in, full output out. points [B, N, 3] fp32 -> [B, npoint] i32."""
    from concourse.bass_utils import run_bass_kernel_spmd

    points = np.ascontiguousarray(np.asarray(points, dtype=np.float32))
    npoint = int(npoint)
    B, N, C = points.shape
    assert C == 3 and N % P == 0
    NF = N // P
    n_cores = 8
    assert B == n_cores, f"expected B == 8, got {B}"

    nc = _get_nc(NF, npoint)
    in_maps = [{"points": points[b].reshape(P, 3 * NF)} for b in range(B)]
    res = run_bass_kernel_spmd(nc, in_maps, list(range(n_cores)))
    out = np.stack(
        [np.asarray(res.results[b]["indices"]).ravel() for b in range(B)]
    ).astype(np.int32)
    return out
